# revision 55
# baseline (speedup 1.0000x reference)
"""GCN 3-layer message-passing kernel for TRN2 (8 NeuronCores, SPMD) — v15.

v5 baseline (2.82 ms) was SWDGE-bound: 1176 dma_gathers + 1176 reg_loads
saturated the Pool engine; the hard wall is SWDGE descriptor generation
(~2.5 ns/descriptor aggregate, 256 B/edge minimum). v15 (2.00 ms):
  - Gathers batched G=4 blocks per call per queue (300 calls), with
    EXACT-call idx packing: per-core edge streams concatenated with no
    per-block slot alignment (only the call tail is 0-padded), cutting
    descriptors 238k -> 211k per layer. A block's matmul schedule covers
    the union (over cores) of slots its edges touch at core-dependent
    offsets; foreign edges in shared slots hit all-zero one-hot columns.
    Compile-time-constant counts (no reg_loads); ~2-3k descriptors/call
    pipelines through the SWDGE rings without head-of-line blocking;
    gather pool bufs=5.
  - Self-loops removed from the gather/scatter path; their contribution
    (the local t-block) rides the existing per-block bias matmul:
    ident @ (b/sqrt(deg) + t_local), bias scaled on the scalar engine.
  - Final projection fused into the L1/L2 transforms (rhs widened to
    [W2 | LW1] / [W3 | LW2]) with an SBUF accumulator; h1T/h2T DRAM
    roundtrip and L3's extra matmuls/loads deleted.
  - Slices [26,26,26,20]: near-equal sizes balance the per-queue gather
    load under the (race-free) 1:1 slice->queue mapping, while the
    smaller last slice still shrinks the inter-layer AllGather bubble.
  - Per-group (512-row) batched DMAs for xT/t-table/t-local/output
    (HWDGE dispatch costs ~0.9 us of issuing-engine time each).
  - L3 log-softmax: Exp in-loop (accumulated row sums), single batched
    Ln + subtract + 2 output DMAs at the end (no act-table thrash).
"""

import os
import sys

sys.path.insert(0, "/opt/trn_rl_repo")

import numpy as np
import ml_dtypes

import concourse.bass as bass  # noqa: F401
import concourse.mybir as mybir
import concourse.tile as tile
from concourse import bacc
from concourse._compat import cdiv
from concourse.bass_utils import run_bass_kernel_spmd

F32 = mybir.dt.float32
F8 = mybir.dt.float8e4
BF16 = mybir.dt.bfloat16
I16 = mybir.dt.int16
I32 = mybir.dt.int32
AL = mybir.AluOpType
AF = mybir.ActivationFunctionType

NC = 8
P = 128
G = 4  # blocks per gather call
SLICE_BLOCKS = [26, 26, 26, 20]  # 98 blocks; balanced queues, smallish final AG
NQ = 4

LAST_EXEC_NS = None
LAST_SCOPES = None


def _cdiv_np(a, b):
    return (a + b - 1) // b


def _group_cumcount(grp: np.ndarray) -> np.ndarray:
    n = len(grp)
    if n == 0:
        return np.zeros(0, dtype=np.int64)
    is_new = np.ones(n, dtype=bool)
    is_new[1:] = grp[1:] != grp[:-1]
    idx = np.arange(n)
    start = np.maximum.accumulate(np.where(is_new, idx, 0))
    return idx - start


def _slice_geometry(S):
    ends = np.cumsum(SLICE_BLOCKS) * P
    starts = np.concatenate([[0], ends[:-1]])
    ends = np.minimum(ends, S)
    rows = ends - starts
    return starts, ends, rows


def _preprocess(edge_index: np.ndarray, n_nodes: int):
    N = n_nodes
    S = N // NC
    n_blocks = cdiv(S, P)
    assert sum(SLICE_BLOCKS) == n_blocks
    n_groups = cdiv(n_blocks, G)
    sstart, send, srows = _slice_geometry(S)
    assert all(r * NC <= 32767 for r in srows)

    src = edge_index[0]
    dst = edge_index[1]
    # deg includes self-loops (reference adds them); edges here exclude them.
    deg = (np.bincount(dst, minlength=N) + 1).astype(np.float64)
    dis = (1.0 / np.sqrt(deg)).astype(np.float32)

    core = dst // S
    block = (dst % S) // P
    t_local = (dst % S) % P

    sc = src // S
    sloc = src % S
    q = np.searchsorted(send, sloc, side="right")
    src_local = sc * srows[q] + (sloc - sstart[q])

    counts = np.zeros((NC, n_blocks, NQ), dtype=np.int64)
    np.add.at(counts, (core, block, q), 1)

    # Exact-call packing: each (group, q) call's idx stream is the per-core
    # concatenation of its blocks' edges with NO per-block slot alignment;
    # only the call tail is 0-padded to a slot boundary. A block's edges can
    # straddle slots at core-dependent offsets, so its matmul schedule covers
    # the UNION (over cores) of touched slots [s_lo, s_hi]; cores without
    # edges in a given slot leave all-zero one-hot columns there.
    ofs = np.zeros((NC, n_blocks, NQ), dtype=np.int64)
    call_slots = np.zeros((n_groups, NQ), dtype=np.int64)
    call_base = np.zeros((n_groups, NQ), dtype=np.int64)  # slot units
    s_lo = np.zeros((n_blocks, NQ), dtype=np.int64)
    s_hi = np.full((n_blocks, NQ), -1, dtype=np.int64)
    acc = 0
    for g in range(n_groups):
        bs = list(range(g * G, min((g + 1) * G, n_blocks)))
        for qq in range(NQ):
            c = counts[:, bs, qq]  # [NC, gb]
            o = np.cumsum(c, axis=1) - c
            ofs[:, bs, qq] = o
            cs = max(1, _cdiv_np(int(c.sum(axis=1).max()), P))
            call_base[g, qq] = acc
            call_slots[g, qq] = cs
            acc += cs
            for i, b in enumerate(bs):
                has = c[:, i] > 0
                if has.any():
                    s_lo[b, qq] = int((o[has, i] // P).min())
                    s_hi[b, qq] = int(((o[has, i] + c[has, i] - 1) // P).max())
    K_total = acc  # gather slots across all calls
    IW = K_total * 8

    ucols = s_hi - s_lo + 1  # [n_blocks, NQ], >= 0
    kb = ucols.sum(axis=1)
    assert (kb >= 1).all()
    toff = np.concatenate([[0], np.cumsum(kb)[:-1]])
    qoff = toff[:, None] + np.concatenate(
        [np.zeros((n_blocks, 1), np.int64), np.cumsum(ucols, axis=1)[:, :-1]], axis=1
    )  # [n_blocks, NQ]
    K_st = int(kb.sum())

    order = np.lexsort((src, q, block, core))
    so_sl = src_local[order]
    so_tl = t_local[order]
    so_core = core[order]
    so_block = block[order]
    so_q = q[order]

    per_core = []
    for c in range(NC):
        m = so_core == c
        cb, cq = so_block[m], so_q[m]
        csl, ctl = so_sl[m], so_tl[m]
        grp = cb * NQ + cq
        pos = _group_cumcount(grp)

        idx16 = np.zeros((16, IW), dtype=np.int16)  # pad = row 0
        tn = np.full((P, K_st), -1.0, dtype=np.float32)  # pad -> zero col

        cpos = ofs[c, cb, cq] + pos  # position within the (g,q) call
        epos = call_base[cb // G, cq] * P + cpos  # global idx entry
        idx16[epos % 16, epos // 16] = csl.astype(np.int16)
        tn[cpos % P, qoff[cb, cq] + cpos // P - s_lo[cb, cq]] = ctl.astype(
            np.float32
        )

        per_core.append(
            {
                "idx16": np.tile(idx16, (8, 1)),
                "tn": tn.astype(ml_dtypes.bfloat16),
            }
        )

    return {
        "s_lo": s_lo,
        "s_hi": s_hi,
        "kb": kb,
        "toff": toff,
        "K_total": K_total,
        "K_st": K_st,
        "IW": IW,
        "call_base": call_base,
        "call_slots": call_slots,
        "n_blocks": n_blocks,
        "n_groups": n_groups,
        "srows": srows,
        "per_core": per_core,
        "dis": dis,
    }


def _build_program(meta, n_nodes, fin, fh, fout):
    N = n_nodes
    S = N // NC
    nb = meta["n_blocks"]
    ng = meta["n_groups"]
    srows = meta["srows"]
    s_lo = meta["s_lo"]
    s_hi = meta["s_hi"]
    kb = meta["kb"]
    toff = meta["toff"]
    K_total = meta["K_total"]
    K_st = meta["K_st"]
    IW = meta["IW"]
    call_base = meta["call_base"]
    call_slots = meta["call_slots"]
    fo_pad = 128
    fx1 = fh + fout  # [W2 | LW1] width
    fx2 = fout + fout  # [W3 | LW2] width
    sstart = np.concatenate([[0], np.cumsum(SLICE_BLOCKS)[:-1]]) * P
    s_end_blocks = np.cumsum(SLICE_BLOCKS) - 1
    max_cs_q = call_slots.max(axis=0)  # per-q dst tile sizes
    # Static greedy queue balancing: slices are uneven (tiny last slice),
    # so assign each call to the least-loaded SWDGE queue instead of 1:1.
    qload = [0] * NQ
    queue_of = {}
    for g in range(ng):
        for qq in range(NQ):
            k = min(range(NQ), key=lambda i: qload[i])
            queue_of[(g, qq)] = k
            qload[k] += int(call_slots[g, qq])

    nc = bacc.Bacc(num_swdge_queues=NQ)

    xT = nc.dram_tensor("xT", [fin, S], BF16, kind="ExternalInput")
    W1 = nc.dram_tensor("W1", [fin, fh], BF16, kind="ExternalInput")
    W2X = nc.dram_tensor("W2X", [fh, fx1], BF16, kind="ExternalInput")
    W3X = nc.dram_tensor("W3X", [fh, fx2], BF16, kind="ExternalInput")
    LW3 = nc.dram_tensor("LW3", [fout, fout], BF16, kind="ExternalInput")
    idx16 = nc.dram_tensor("idx16", [P, IW], I16, kind="ExternalInput")
    tn_in = nc.dram_tensor("tn", [P, K_st], BF16, kind="ExternalInput")
    iota_in = nc.dram_tensor("iota", [P, P], BF16, kind="ExternalInput")
    ident_in = nc.dram_tensor("ident", [P, P], BF16, kind="ExternalInput")
    dis_in = nc.dram_tensor("disblk", [P, nb], F32, kind="ExternalInput")
    dinv_in = nc.dram_tensor("dinvblk", [P, nb], F32, kind="ExternalInput")
    bb1_in = nc.dram_tensor("bb1", [P, fh], BF16, kind="ExternalInput")
    bb2_in = nc.dram_tensor("bb2", [P, fh], BF16, kind="ExternalInput")
    bb3_in = nc.dram_tensor("bb3", [P, fout], BF16, kind="ExternalInput")
    lbbc_in = nc.dram_tensor("lbbc", [P, fout], BF16, kind="ExternalInput")
    out_sh = nc.dram_tensor("out_sh", [S, fout], F32, kind="ExternalOutput")

    t1_sh = nc.dram_tensor("t1_sh", [S, fh], F8)
    t2_sh = nc.dram_tensor("t2_sh", [S, fh], F8)
    t3_sh = nc.dram_tensor("t3_sh", [S, fo_pad], BF16)

    tabs = {}
    tab_dt = {1: F8, 2: F8, 3: BF16}
    for li, fel in ((1, fh), (2, fh), (3, fo_pad)):
        tabs[li] = tuple(
            nc.dram_tensor(f"t{li}S{s}", [int(srows[s]) * NC, fel], tab_dt[li],
                           addr_space="Shared")
            for s in range(NQ)
        )
    t_shs = {1: t1_sh, 2: t2_sh, 3: t3_sh}

    rg = [list(range(NC))]

    def used_rows(b):
        return min(P, S - b * P)

    def fire_ag(li, s):
        t_sh = t_shs[li]
        r0, r1 = int(sstart[s]), int(sstart[s]) + int(srows[s])
        sc = nc.enter_named_scope(f"ag{li}S{s}", False)
        nc.gpsimd.collective_compute(
            "AllGather", AL.bypass, ins=[t_sh[r0:r1, :]], outs=[tabs[li][s][:, :]],
            replica_groups=rg,
        )
        nc.leave_named_scope(f"ag{li}S{s}", sc[0], False)

    with tile.TileContext(nc) as tc:
        with (
            tc.tile_pool(name="const", bufs=1) as cpool,
            tc.tile_pool(name="sb", bufs=3) as pool,
            tc.tile_pool(name="gath", bufs=5) as gpool,
            tc.tile_pool(name="st", bufs=4) as stpool,
            tc.tile_pool(name="xg", bufs=2) as xgpool,
            tc.tile_pool(name="ps", bufs=2, space="PSUM") as psp,
        ):
            iota_t = cpool.tile([P, P], BF16)
            nc.sync.dma_start(out=iota_t[:], in_=iota_in[:, :])
            ident_t = cpool.tile([P, P], BF16)
            nc.sync.dma_start(out=ident_t[:], in_=ident_in[:, :])
            w1_t = cpool.tile([P, 2, fh], BF16)
            nc.sync.dma_start(out=w1_t[:], in_=W1[:, :].rearrange("(c k) f -> k c f", k=P))
            w2x_t = cpool.tile([P, 2, fx1], BF16)
            nc.sync.dma_start(out=w2x_t[:], in_=W2X[:, :].rearrange("(c k) f -> k c f", k=P))
            w3x_t = cpool.tile([P, 2, fx2], BF16)
            nc.sync.dma_start(out=w3x_t[:], in_=W3X[:, :].rearrange("(c k) f -> k c f", k=P))
            lw3_t = cpool.tile([fout, fout], BF16)
            nc.sync.dma_start(out=lw3_t[:], in_=LW3[:, :])
            lbbc = cpool.tile([P, fout], BF16)
            nc.sync.dma_start(out=lbbc[:], in_=lbbc_in[:, :])
            dis_t = cpool.tile([P, nb], F32)
            nc.sync.dma_start(out=dis_t[:], in_=dis_in[:, :])
            dinv_t = cpool.tile([P, nb], F32)
            nc.sync.dma_start(out=dinv_t[:], in_=dinv_in[:, :])
            bb_ts = {}
            for li, (bb_in, fel) in enumerate(
                ((bb1_in, fh), (bb2_in, fh), (bb3_in, fout)), start=1
            ):
                bb_t = cpool.tile([P, fel], BF16)
                nc.sync.dma_start(out=bb_t[:], in_=bb_in[:, :])
                bb_ts[li] = bb_t
            idx_t = cpool.tile([P, IW], I16)
            nc.sync.dma_start(out=idx_t[:], in_=idx16[:, :])
            tn_t = cpool.tile([P, K_st], BF16)
            nc.sync.dma_start(out=tn_t[:], in_=tn_in[:, :])
            acc_t = cpool.tile([P, nb, fout], F32)  # fused projection accum
            zf = cpool.tile([P, nb, fout], F32)  # final log-probs
            mbuf = cpool.tile([P, nb], F32)
            ssbuf = cpool.tile([P, nb], F32)
            nc.vector.memset(mbuf[:], 0.0)
            nc.vector.memset(ssbuf[:], 1.0)

            sc_T = nc.enter_named_scope("phaseT", False)
            for g in range(ng):
                b0 = g * G
                gb = min(G, nb - b0)
                gw = min(G * P, S - b0 * P)
                xg = xgpool.tile([P, 2, G * P], BF16, tag="xg")
                for cc in range(2):
                    nc.sync.dma_start(
                        out=xg[:, cc, :gw],
                        in_=xT[cc * P : (cc + 1) * P, b0 * P : b0 * P + gw],
                    )
                ev_g = pool.tile([P, G, fh], F8, tag="evq")
                for bi in range(gb):
                    b = b0 + bi
                    u = used_rows(b)
                    ps1 = psp.tile([P, fh], F32, tag="ps2")
                    for cc in range(2):
                        nc.tensor.matmul(
                            ps1[:u, :],
                            lhsT=xg[:, cc, bi * P : bi * P + u],
                            rhs=w1_t[:, cc, :],
                            start=(cc == 0),
                            stop=(cc == 1),
                        )
                    nc.scalar.activation(
                        ev_g[:u, bi, :], ps1[:u, :], AF.Copy,
                        scale=dis_t[:u, b : b + 1],
                    )
                if gw == gb * P:
                    nc.sync.dma_start(
                        out=t1_sh[b0 * P : b0 * P + gw, :].rearrange(
                            "(c k) f -> k c f", k=P
                        ),
                        in_=ev_g[:, :gb, :],
                    )
                else:
                    for bi in range(gb):
                        uu = used_rows(b0 + bi)
                        nc.sync.dma_start(
                            out=t1_sh[(b0 + bi) * P : (b0 + bi) * P + uu, :],
                            in_=ev_g[:uu, bi, :],
                        )
                for s in range(NQ):
                    if b0 <= int(s_end_blocks[s]) < b0 + gb:
                        fire_ag(1, s)
            nc.leave_named_scope("phaseT", sc_T[0], False)

            def layer(li, felem):
                fagg = fh if li < 3 else fout
                g_dt = F8 if li < 3 else BF16
                bb_t = bb_ts[li]
                t_sh = t_shs[li]
                for g in range(ng):
                    b0 = g * G
                    gb = min(G, nb - b0)
                    dsts = []
                    for qq in range(NQ):
                        cs = int(call_slots[g, qq])
                        cbase = int(call_base[g, qq])
                        tab = tabs[li][qq]
                        rows = int(srows[qq]) * NC
                        dst = gpool.tile(
                            [P, int(max_cs_q[qq]), felem], g_dt, tag=f"dst{qq}"
                        )
                        nc.gpsimd.dma_gather(
                            dst[:, :cs, :],
                            tab[0:rows, :],
                            idx_t[:, cbase * 8 : (cbase + cs) * 8],
                            cs * P,
                            cs * P,
                            felem,
                            single_packet=False,
                            queue_num=queue_of[(g, qq)],
                        )
                        dsts.append(dst)

                    gw = min(G * P, S - b0 * P)
                    tl_g = pool.tile([P, G, fagg], g_dt, tag="tl")
                    if gw == gb * P:
                        nc.scalar.dma_start(
                            out=tl_g[:, :gb, :],
                            in_=t_sh[b0 * P : b0 * P + gw, :fagg].rearrange(
                                "(c k) f -> k c f", k=P
                            ),
                        )
                    else:
                        for bi in range(gb):
                            uu = used_rows(b0 + bi)
                            nc.scalar.dma_start(
                                out=tl_g[:uu, bi, :],
                                in_=t_sh[(b0 + bi) * P : (b0 + bi) * P + uu, :fagg],
                            )
                    if li < 3:
                        fnext = fh if li == 1 else fout
                        ev_dt = F8 if li == 1 else BF16
                        tnext = t2_sh if li == 1 else t3_sh
                        ev_g = pool.tile([P, G, fnext], ev_dt, tag="ev")

                    for bi in range(gb):
                        b = b0 + bi
                        u = used_rows(b)
                        kbb = int(kb[b])
                        soff = int(toff[b])

                        st_t = stpool.tile([P, kbb, P], g_dt, tag="st")
                        in0 = iota_t[:, :].unsqueeze(1).broadcast_to([P, kbb, P])
                        in1 = (
                            tn_t[:, soff : soff + kbb]
                            .unsqueeze(2)
                            .broadcast_to([P, kbb, P])
                        )
                        nc.vector.tensor_tensor(
                            out=st_t[:, :, :], in0=in0, in1=in1, op=AL.is_equal
                        )

                        # bias + self-loop: ident @ (b/deg^.5 + t_local_block)
                        bdt = pool.tile([P, fagg], BF16, tag="bdt")
                        nc.scalar.activation(
                            bdt[:, :], bb_t[:, :], AF.Copy,
                            scale=dinv_t[:, b : b + 1],
                        )
                        tlp = pool.tile([P, fagg], BF16, tag="tlp")
                        nc.vector.tensor_tensor(
                            out=tlp[:u, :], in0=tl_g[:u, bi, :], in1=bdt[:u, :],
                            op=AL.add,
                        )

                        psa = psp.tile([P, fagg], F32, tag="psa")
                        s = 0
                        for qq in range(NQ):
                            for sl in range(int(s_lo[b, qq]), int(s_hi[b, qq]) + 1):
                                nc.tensor.matmul(
                                    psa[:],
                                    lhsT=st_t[:, s, :],
                                    rhs=dsts[qq][:, sl, :fagg],
                                    start=(s == 0),
                                    stop=False,
                                )
                                s += 1
                        nc.tensor.matmul(
                            psa[:], lhsT=ident_t[:], rhs=tlp[:, :],
                            start=False, stop=True,
                        )
                        h_sb = pool.tile([P, fagg], BF16, tag="h_sb")
                        nc.scalar.activation(
                            h_sb[:u, :], psa[:u, :], AF.Relu, scale=dis_t[:u, b : b + 1]
                        )

                        if li < 3:
                            wx = w2x_t if li == 1 else w3x_t
                            fxw = fx1 if li == 1 else fx2
                            ps2 = psp.tile([P, fxw], F32, tag="ps2")
                            hT2 = pool.tile([P, 2, P], BF16, tag="hT2")
                            for cc in range(2):
                                pst = psp.tile([P, P], BF16, tag=f"pst{cc}")
                                nc.tensor.transpose(
                                    pst[:], h_sb[:, cc * P : (cc + 1) * P], ident_t[:]
                                )
                                nc.vector.tensor_copy(hT2[:, cc, :], pst[:])
                                nc.tensor.matmul(
                                    ps2[:u, :],
                                    lhsT=hT2[:, cc, :u],
                                    rhs=wx[:, cc, :fxw],
                                    start=(cc == 0),
                                    stop=(cc == 1),
                                )
                            nc.scalar.activation(
                                ev_g[:u, bi, :], ps2[:u, :fnext], AF.Copy,
                                scale=dis_t[:u, b : b + 1],
                            )
                            # fused projection partial: h{li} @ LW{li}
                            if li == 1:
                                nc.vector.tensor_tensor(
                                    out=acc_t[:u, b, :], in0=ps2[:u, fh:fx1],
                                    in1=lbbc[:u, :], op=AL.add,
                                )
                            else:
                                nc.vector.tensor_tensor(
                                    out=acc_t[:u, b, :], in0=ps2[:u, fout:fx2],
                                    in1=acc_t[:u, b, :], op=AL.add,
                                )
                        else:
                            ps3t = psp.tile([P, P], BF16, tag="pst0")
                            nc.tensor.transpose(ps3t[:fout, :], h_sb[:, :fout], ident_t[:])
                            h3T = pool.tile([fout, P], BF16, tag="hT0")
                            nc.vector.tensor_copy(h3T[:], ps3t[:fout, :])
                            pso = psp.tile([P, fout], F32, tag="ps2")
                            nc.tensor.matmul(
                                pso[:u, :], lhsT=h3T[:, :u], rhs=lw3_t[:, :],
                                start=True, stop=True,
                            )
                            # z = pso + acc (concat proj complete), kept in acc
                            nc.vector.tensor_tensor(
                                out=acc_t[:u, b, :], in0=pso[:u, :],
                                in1=acc_t[:u, b, :], op=AL.add,
                            )
                            nc.vector.tensor_reduce(
                                mbuf[:u, b : b + 1], acc_t[:u, b, :],
                                mybir.AxisListType.X, AL.max,
                            )
                            nm = pool.tile([P, 1], F32, tag="nm")
                            nc.vector.tensor_scalar(
                                out=nm[:u, :], in0=mbuf[:u, b : b + 1],
                                scalar1=-1.0, scalar2=None, op0=AL.mult,
                            )
                            e_t = pool.tile([P, fout], F32, tag="e_t")
                            nc.scalar.activation(
                                e_t[:u, :], acc_t[:u, b, :], AF.Exp,
                                bias=nm[:u, :1],
                                accum_out=ssbuf[:u, b : b + 1],
                            )

                    if li < 3:
                        if gw == gb * P:
                            nc.sync.dma_start(
                                out=tnext[b0 * P : b0 * P + gw, :fnext].rearrange(
                                    "(c k) f -> k c f", k=P
                                ),
                                in_=ev_g[:, :gb, :],
                            )
                        else:
                            for bi in range(gb):
                                uu = used_rows(b0 + bi)
                                nc.sync.dma_start(
                                    out=tnext[
                                        (b0 + bi) * P : (b0 + bi) * P + uu, :fnext
                                    ],
                                    in_=ev_g[:uu, bi, :],
                                )
                        for s4 in range(NQ):
                            if b0 <= int(s_end_blocks[s4]) < b0 + gb:
                                fire_ag(li + 1, s4)

                if li == 3:
                    ls_t = pool.tile([P, nb], F32, tag="ls_t")
                    nc.scalar.activation(ls_t[:, :], ssbuf[:, :], AF.Ln)
                    mls = pool.tile([P, nb], F32, tag="mls")
                    nc.vector.tensor_tensor(
                        out=mls[:, :], in0=mbuf[:, :], in1=ls_t[:, :], op=AL.add
                    )
                    nc.vector.tensor_tensor(
                        out=zf[:, :, :], in0=acc_t[:, :, :],
                        in1=mls[:, :].unsqueeze(2).broadcast_to([P, nb, fout]),
                        op=AL.subtract,
                    )
                    nc.sync.dma_start(
                        out=out_sh[0 : (nb - 1) * P, :].rearrange(
                            "(c k) f -> k c f", k=P
                        ),
                        in_=zf[:, : nb - 1, :],
                    )
                    lastu = S - (nb - 1) * P
                    nc.sync.dma_start(
                        out=out_sh[(nb - 1) * P :, :], in_=zf[:lastu, nb - 1, :]
                    )

            sc = nc.enter_named_scope("L1", False)
            layer(1, fh)
            nc.leave_named_scope("L1", sc[0], False)
            sc = nc.enter_named_scope("L2", False)
            layer(2, fh)
            nc.leave_named_scope("L2", sc[0], False)
            sc = nc.enter_named_scope("L3", False)
            layer(3, fo_pad)
            nc.leave_named_scope("L3", sc[0], False)

    nc.finalize()
    return nc


def kernel(x, edge_index, W1, b1, W2, b2, W3, b3, lin_w, lin_b):
    global LAST_EXEC_NS, LAST_SCOPES
    x = np.asarray(x)
    N = x.shape[0]
    S = N // NC
    fin, fh, fout = W1.shape[0], W2.shape[0], W3.shape[1]

    meta = _preprocess(np.asarray(edge_index, dtype=np.int64), N)
    nc = _build_program(meta, N, fin, fh, fout)

    dis = meta["dis"]
    nb = meta["n_blocks"]

    iota = np.tile(np.arange(P, dtype=np.float32), (P, 1)).astype(ml_dtypes.bfloat16)
    ident = np.eye(P, dtype=np.float32).astype(ml_dtypes.bfloat16)
    lbbc = np.tile(np.asarray(lin_b, np.float32), (P, 1)).astype(ml_dtypes.bfloat16)
    bb1 = np.tile(np.asarray(b1, np.float32), (P, 1)).astype(ml_dtypes.bfloat16)
    bb2 = np.tile(np.asarray(b2, np.float32), (P, 1)).astype(ml_dtypes.bfloat16)
    bb3 = np.tile(np.asarray(b3, np.float32), (P, 1)).astype(ml_dtypes.bfloat16)
    lw = np.asarray(lin_w, np.float32)
    w2x = np.concatenate([np.asarray(W2, np.float32), lw[:fh]], axis=1)
    w3x = np.concatenate([np.asarray(W3, np.float32), lw[fh : 2 * fh]], axis=1)
    lw3 = lw[2 * fh :]

    in_maps = []
    for c in range(NC):
        xs = np.asarray(x[c * S : (c + 1) * S], np.float32)
        dc = dis[c * S : (c + 1) * S]
        dis_blk = np.ones((P, nb), dtype=np.float32)
        for b in range(nb):
            u = min(P, S - b * P)
            dis_blk[:u, b] = dc[b * P : b * P + u]
        in_maps.append(
            {
                "xT": np.ascontiguousarray(xs.T).astype(ml_dtypes.bfloat16),
                "W1": np.asarray(W1, np.float32).astype(ml_dtypes.bfloat16),
                "W2X": w2x.astype(ml_dtypes.bfloat16),
                "W3X": w3x.astype(ml_dtypes.bfloat16),
                "LW3": lw3.astype(ml_dtypes.bfloat16),
                "idx16": meta["per_core"][c]["idx16"],
                "tn": meta["per_core"][c]["tn"],
                "iota": iota,
                "ident": ident,
                "disblk": dis_blk,
                "dinvblk": 1.0 / dis_blk,
                "bb1": bb1,
                "bb2": bb2,
                "bb3": bb3,
                "lbbc": lbbc,
            }
        )
    trace = bool(os.environ.get("GCN_TRACE"))
    res = run_bass_kernel_spmd(nc, in_maps, list(range(NC)), trace=trace)
    LAST_EXEC_NS = res.exec_time_ns
    LAST_SCOPES = res.per_core_scope_times
    out = np.concatenate([res.results[c]["out_sh"] for c in range(NC)], axis=0)
    return out.astype(np.float32)


# revision 56
# speedup vs baseline: 1.0182x; 1.0182x over previous
"""GCN 3-layer message-passing kernel for TRN2 (8 NeuronCores, SPMD) — v18.

v5 baseline (2.82 ms) was SWDGE-bound: 1176 dma_gathers + 1176 reg_loads
saturated the Pool engine; the hard wall is SWDGE descriptor generation
(~2.5 ns/descriptor aggregate, 256 B/edge minimum). v18 (1.92 ms):
  - Gathers batched G=4 blocks per call per queue (300 calls), with
    EXACT-call idx packing: per-core edge streams concatenated with no
    per-block slot alignment (only the call tail is 0-padded), cutting
    descriptors 238k -> 211k per layer. A block's matmul schedule covers
    the union (over cores) of slots its edges touch at core-dependent
    offsets; foreign edges in shared slots hit all-zero one-hot columns.
    Compile-time-constant counts (no reg_loads); ~2-3k descriptors/call
    pipelines through the SWDGE rings without head-of-line blocking;
    gather pool bufs=5, st-build pool bufs=4 (pre-builds one-hot tiles
    through AllGather-gated stalls).
  - Self-loops removed from the gather/scatter path; their contribution
    (the local t-block) rides the existing per-block bias matmul:
    ident @ (b/sqrt(deg) + t_local), bias scaled on the scalar engine.
  - Final projection fused into the L1/L2 transforms (rhs widened to
    [W2 | LW1] / [W3 | LW2]) with an SBUF accumulator; h1T/h2T DRAM
    roundtrip and L3's extra matmuls/loads deleted.
  - Slices [26,26,26,20]: near-equal sizes balance the per-queue gather
    load under the (race-free) 1:1 slice->queue mapping, while the
    smaller last slice still shrinks the inter-layer AllGather bubble.
  - Per-group (512-row) batched DMAs for xT/t-table/t-local/output
    (HWDGE dispatch costs ~0.9 us of issuing-engine time each).
  - L3 log-softmax: Exp in-loop (accumulated row sums), single batched
    Ln + subtract + 2 output DMAs at the end (no act-table thrash).
"""

import os
import sys

sys.path.insert(0, "/opt/trn_rl_repo")

import numpy as np
import ml_dtypes

import concourse.bass as bass  # noqa: F401
import concourse.mybir as mybir
import concourse.tile as tile
from concourse import bacc
from concourse._compat import cdiv
from concourse.bass_utils import run_bass_kernel_spmd

F32 = mybir.dt.float32
F8 = mybir.dt.float8e4
BF16 = mybir.dt.bfloat16
I16 = mybir.dt.int16
I32 = mybir.dt.int32
AL = mybir.AluOpType
AF = mybir.ActivationFunctionType

NC = 8
P = 128
G = 4  # blocks per gather call
SLICE_BLOCKS = [26, 26, 26, 20]  # 98 blocks; balanced queues, smallish final AG
NQ = 4

LAST_EXEC_NS = None
LAST_SCOPES = None


def _cdiv_np(a, b):
    return (a + b - 1) // b


def _group_cumcount(grp: np.ndarray) -> np.ndarray:
    n = len(grp)
    if n == 0:
        return np.zeros(0, dtype=np.int64)
    is_new = np.ones(n, dtype=bool)
    is_new[1:] = grp[1:] != grp[:-1]
    idx = np.arange(n)
    start = np.maximum.accumulate(np.where(is_new, idx, 0))
    return idx - start


def _slice_geometry(S):
    ends = np.cumsum(SLICE_BLOCKS) * P
    starts = np.concatenate([[0], ends[:-1]])
    ends = np.minimum(ends, S)
    rows = ends - starts
    return starts, ends, rows


def _preprocess(edge_index: np.ndarray, n_nodes: int):
    N = n_nodes
    S = N // NC
    n_blocks = cdiv(S, P)
    assert sum(SLICE_BLOCKS) == n_blocks
    n_groups = cdiv(n_blocks, G)
    sstart, send, srows = _slice_geometry(S)
    assert all(r * NC <= 32767 for r in srows)

    src = edge_index[0]
    dst = edge_index[1]
    # deg includes self-loops (reference adds them); edges here exclude them.
    deg = (np.bincount(dst, minlength=N) + 1).astype(np.float64)
    dis = (1.0 / np.sqrt(deg)).astype(np.float32)

    core = dst // S
    block = (dst % S) // P
    t_local = (dst % S) % P

    sc = src // S
    sloc = src % S
    q = np.searchsorted(send, sloc, side="right")
    src_local = sc * srows[q] + (sloc - sstart[q])

    counts = np.zeros((NC, n_blocks, NQ), dtype=np.int64)
    np.add.at(counts, (core, block, q), 1)

    # Exact-call packing: each (group, q) call's idx stream is the per-core
    # concatenation of its blocks' edges with NO per-block slot alignment;
    # only the call tail is 0-padded to a slot boundary. A block's edges can
    # straddle slots at core-dependent offsets, so its matmul schedule covers
    # the UNION (over cores) of touched slots [s_lo, s_hi]; cores without
    # edges in a given slot leave all-zero one-hot columns there.
    ofs = np.zeros((NC, n_blocks, NQ), dtype=np.int64)
    call_slots = np.zeros((n_groups, NQ), dtype=np.int64)
    call_base = np.zeros((n_groups, NQ), dtype=np.int64)  # slot units
    s_lo = np.zeros((n_blocks, NQ), dtype=np.int64)
    s_hi = np.full((n_blocks, NQ), -1, dtype=np.int64)
    acc = 0
    for g in range(n_groups):
        bs = list(range(g * G, min((g + 1) * G, n_blocks)))
        for qq in range(NQ):
            c = counts[:, bs, qq]  # [NC, gb]
            o = np.cumsum(c, axis=1) - c
            ofs[:, bs, qq] = o
            cs = max(1, _cdiv_np(int(c.sum(axis=1).max()), P))
            call_base[g, qq] = acc
            call_slots[g, qq] = cs
            acc += cs
            for i, b in enumerate(bs):
                has = c[:, i] > 0
                if has.any():
                    s_lo[b, qq] = int((o[has, i] // P).min())
                    s_hi[b, qq] = int(((o[has, i] + c[has, i] - 1) // P).max())
    K_total = acc  # gather slots across all calls
    IW = K_total * 8

    ucols = s_hi - s_lo + 1  # [n_blocks, NQ], >= 0
    kb = ucols.sum(axis=1)
    assert (kb >= 1).all()
    toff = np.concatenate([[0], np.cumsum(kb)[:-1]])
    qoff = toff[:, None] + np.concatenate(
        [np.zeros((n_blocks, 1), np.int64), np.cumsum(ucols, axis=1)[:, :-1]], axis=1
    )  # [n_blocks, NQ]
    K_st = int(kb.sum())

    order = np.lexsort((src, q, block, core))
    so_sl = src_local[order]
    so_tl = t_local[order]
    so_core = core[order]
    so_block = block[order]
    so_q = q[order]

    per_core = []
    for c in range(NC):
        m = so_core == c
        cb, cq = so_block[m], so_q[m]
        csl, ctl = so_sl[m], so_tl[m]
        grp = cb * NQ + cq
        pos = _group_cumcount(grp)

        idx16 = np.zeros((16, IW), dtype=np.int16)  # pad = row 0
        tn = np.full((P, K_st), -1.0, dtype=np.float32)  # pad -> zero col

        cpos = ofs[c, cb, cq] + pos  # position within the (g,q) call
        epos = call_base[cb // G, cq] * P + cpos  # global idx entry
        idx16[epos % 16, epos // 16] = csl.astype(np.int16)
        tn[cpos % P, qoff[cb, cq] + cpos // P - s_lo[cb, cq]] = ctl.astype(
            np.float32
        )

        per_core.append(
            {
                "idx16": np.tile(idx16, (8, 1)),
                "tn": tn.astype(ml_dtypes.bfloat16),
            }
        )

    return {
        "s_lo": s_lo,
        "s_hi": s_hi,
        "kb": kb,
        "toff": toff,
        "K_total": K_total,
        "K_st": K_st,
        "IW": IW,
        "call_base": call_base,
        "call_slots": call_slots,
        "n_blocks": n_blocks,
        "n_groups": n_groups,
        "srows": srows,
        "per_core": per_core,
        "dis": dis,
    }


def _build_program(meta, n_nodes, fin, fh, fout):
    N = n_nodes
    S = N // NC
    nb = meta["n_blocks"]
    ng = meta["n_groups"]
    srows = meta["srows"]
    s_lo = meta["s_lo"]
    s_hi = meta["s_hi"]
    kb = meta["kb"]
    toff = meta["toff"]
    K_total = meta["K_total"]
    K_st = meta["K_st"]
    IW = meta["IW"]
    call_base = meta["call_base"]
    call_slots = meta["call_slots"]
    fo_pad = 128
    fx1 = fh + fout  # [W2 | LW1] width
    fx2 = fout + fout  # [W3 | LW2] width
    sstart = np.concatenate([[0], np.cumsum(SLICE_BLOCKS)[:-1]]) * P
    s_end_blocks = np.cumsum(SLICE_BLOCKS) - 1
    max_cs_q = call_slots.max(axis=0)  # per-q dst tile sizes
    # Static greedy queue balancing: slices are uneven (tiny last slice),
    # so assign each call to the least-loaded SWDGE queue instead of 1:1.
    qload = [0] * NQ
    queue_of = {}
    for g in range(ng):
        for qq in range(NQ):
            k = min(range(NQ), key=lambda i: qload[i])
            queue_of[(g, qq)] = k
            qload[k] += int(call_slots[g, qq])

    nc = bacc.Bacc(num_swdge_queues=NQ)

    xT = nc.dram_tensor("xT", [fin, S], BF16, kind="ExternalInput")
    W1 = nc.dram_tensor("W1", [fin, fh], BF16, kind="ExternalInput")
    W2X = nc.dram_tensor("W2X", [fh, fx1], BF16, kind="ExternalInput")
    W3X = nc.dram_tensor("W3X", [fh, fx2], BF16, kind="ExternalInput")
    LW3 = nc.dram_tensor("LW3", [fout, fout], BF16, kind="ExternalInput")
    idx16 = nc.dram_tensor("idx16", [P, IW], I16, kind="ExternalInput")
    tn_in = nc.dram_tensor("tn", [P, K_st], BF16, kind="ExternalInput")
    iota_in = nc.dram_tensor("iota", [P, P], BF16, kind="ExternalInput")
    ident_in = nc.dram_tensor("ident", [P, P], BF16, kind="ExternalInput")
    dis_in = nc.dram_tensor("disblk", [P, nb], F32, kind="ExternalInput")
    dinv_in = nc.dram_tensor("dinvblk", [P, nb], F32, kind="ExternalInput")
    bb1_in = nc.dram_tensor("bb1", [P, fh], BF16, kind="ExternalInput")
    bb2_in = nc.dram_tensor("bb2", [P, fh], BF16, kind="ExternalInput")
    bb3_in = nc.dram_tensor("bb3", [P, fout], BF16, kind="ExternalInput")
    lbbc_in = nc.dram_tensor("lbbc", [P, fout], BF16, kind="ExternalInput")
    out_sh = nc.dram_tensor("out_sh", [S, fout], F32, kind="ExternalOutput")

    t1_sh = nc.dram_tensor("t1_sh", [S, fh], F8)
    t2_sh = nc.dram_tensor("t2_sh", [S, fh], F8)
    t3_sh = nc.dram_tensor("t3_sh", [S, fo_pad], BF16)

    tabs = {}
    tab_dt = {1: F8, 2: F8, 3: BF16}
    for li, fel in ((1, fh), (2, fh), (3, fo_pad)):
        tabs[li] = tuple(
            nc.dram_tensor(f"t{li}S{s}", [int(srows[s]) * NC, fel], tab_dt[li],
                           addr_space="Shared")
            for s in range(NQ)
        )
    t_shs = {1: t1_sh, 2: t2_sh, 3: t3_sh}

    rg = [list(range(NC))]

    def used_rows(b):
        return min(P, S - b * P)

    def fire_ag(li, s):
        t_sh = t_shs[li]
        r0, r1 = int(sstart[s]), int(sstart[s]) + int(srows[s])
        sc = nc.enter_named_scope(f"ag{li}S{s}", False)
        nc.gpsimd.collective_compute(
            "AllGather", AL.bypass, ins=[t_sh[r0:r1, :]], outs=[tabs[li][s][:, :]],
            replica_groups=rg,
        )
        nc.leave_named_scope(f"ag{li}S{s}", sc[0], False)

    with tile.TileContext(nc) as tc:
        with (
            tc.tile_pool(name="const", bufs=1) as cpool,
            tc.tile_pool(name="sb", bufs=3) as pool,
            tc.tile_pool(name="gath", bufs=5) as gpool,
            tc.tile_pool(name="st", bufs=4) as stpool,
            tc.tile_pool(name="xg", bufs=2) as xgpool,
            tc.tile_pool(name="ps", bufs=2, space="PSUM") as psp,
        ):
            iota_t = cpool.tile([P, P], BF16)
            nc.sync.dma_start(out=iota_t[:], in_=iota_in[:, :])
            ident_t = cpool.tile([P, P], BF16)
            nc.sync.dma_start(out=ident_t[:], in_=ident_in[:, :])
            w1_t = cpool.tile([P, 2, fh], BF16)
            nc.sync.dma_start(out=w1_t[:], in_=W1[:, :].rearrange("(c k) f -> k c f", k=P))
            w2x_t = cpool.tile([P, 2, fx1], BF16)
            nc.sync.dma_start(out=w2x_t[:], in_=W2X[:, :].rearrange("(c k) f -> k c f", k=P))
            w3x_t = cpool.tile([P, 2, fx2], BF16)
            nc.sync.dma_start(out=w3x_t[:], in_=W3X[:, :].rearrange("(c k) f -> k c f", k=P))
            lw3_t = cpool.tile([fout, fout], BF16)
            nc.sync.dma_start(out=lw3_t[:], in_=LW3[:, :])
            lbbc = cpool.tile([P, fout], BF16)
            nc.sync.dma_start(out=lbbc[:], in_=lbbc_in[:, :])
            dis_t = cpool.tile([P, nb], F32)
            nc.sync.dma_start(out=dis_t[:], in_=dis_in[:, :])
            dinv_t = cpool.tile([P, nb], F32)
            nc.sync.dma_start(out=dinv_t[:], in_=dinv_in[:, :])
            bb_ts = {}
            for li, (bb_in, fel) in enumerate(
                ((bb1_in, fh), (bb2_in, fh), (bb3_in, fout)), start=1
            ):
                bb_t = cpool.tile([P, fel], BF16)
                nc.sync.dma_start(out=bb_t[:], in_=bb_in[:, :])
                bb_ts[li] = bb_t
            idx_t = cpool.tile([P, IW], I16)
            nc.sync.dma_start(out=idx_t[:], in_=idx16[:, :])
            tn_t = cpool.tile([P, K_st], BF16)
            nc.sync.dma_start(out=tn_t[:], in_=tn_in[:, :])
            acc_t = cpool.tile([P, nb, fout], F32)  # fused projection accum
            zf = cpool.tile([P, nb, fout], F32)  # final log-probs
            mbuf = cpool.tile([P, nb], F32)
            ssbuf = cpool.tile([P, nb], F32)
            nc.vector.memset(mbuf[:], 0.0)
            nc.vector.memset(ssbuf[:], 1.0)

            sc_T = nc.enter_named_scope("phaseT", False)
            for g in range(ng):
                b0 = g * G
                gb = min(G, nb - b0)
                gw = min(G * P, S - b0 * P)
                xg = xgpool.tile([P, 2, G * P], BF16, tag="xg")
                for cc in range(2):
                    nc.sync.dma_start(
                        out=xg[:, cc, :gw],
                        in_=xT[cc * P : (cc + 1) * P, b0 * P : b0 * P + gw],
                    )
                ev_g = pool.tile([P, G, fh], F8, tag="evq")
                for bi in range(gb):
                    b = b0 + bi
                    u = used_rows(b)
                    ps1 = psp.tile([P, fh], F32, tag="ps2")
                    for cc in range(2):
                        nc.tensor.matmul(
                            ps1[:u, :],
                            lhsT=xg[:, cc, bi * P : bi * P + u],
                            rhs=w1_t[:, cc, :],
                            start=(cc == 0),
                            stop=(cc == 1),
                        )
                    nc.scalar.activation(
                        ev_g[:u, bi, :], ps1[:u, :], AF.Copy,
                        scale=dis_t[:u, b : b + 1],
                    )
                if gw == gb * P:
                    nc.sync.dma_start(
                        out=t1_sh[b0 * P : b0 * P + gw, :].rearrange(
                            "(c k) f -> k c f", k=P
                        ),
                        in_=ev_g[:, :gb, :],
                    )
                else:
                    for bi in range(gb):
                        uu = used_rows(b0 + bi)
                        nc.sync.dma_start(
                            out=t1_sh[(b0 + bi) * P : (b0 + bi) * P + uu, :],
                            in_=ev_g[:uu, bi, :],
                        )
                for s in range(NQ):
                    if b0 <= int(s_end_blocks[s]) < b0 + gb:
                        fire_ag(1, s)
            nc.leave_named_scope("phaseT", sc_T[0], False)

            def layer(li, felem):
                fagg = fh if li < 3 else fout
                g_dt = F8 if li < 3 else BF16
                bb_t = bb_ts[li]
                t_sh = t_shs[li]
                for g in range(ng):
                    b0 = g * G
                    gb = min(G, nb - b0)
                    dsts = []
                    for qq in range(NQ):
                        cs = int(call_slots[g, qq])
                        cbase = int(call_base[g, qq])
                        tab = tabs[li][qq]
                        rows = int(srows[qq]) * NC
                        dst = gpool.tile(
                            [P, int(max_cs_q[qq]), felem], g_dt, tag=f"dst{qq}"
                        )
                        nc.gpsimd.dma_gather(
                            dst[:, :cs, :],
                            tab[0:rows, :],
                            idx_t[:, cbase * 8 : (cbase + cs) * 8],
                            cs * P,
                            cs * P,
                            felem,
                            single_packet=False,
                            queue_num=queue_of[(g, qq)],
                        )
                        dsts.append(dst)

                    gw = min(G * P, S - b0 * P)
                    tl_g = pool.tile([P, G, fagg], g_dt, tag="tl")
                    if gw == gb * P:
                        nc.scalar.dma_start(
                            out=tl_g[:, :gb, :],
                            in_=t_sh[b0 * P : b0 * P + gw, :fagg].rearrange(
                                "(c k) f -> k c f", k=P
                            ),
                        )
                    else:
                        for bi in range(gb):
                            uu = used_rows(b0 + bi)
                            nc.scalar.dma_start(
                                out=tl_g[:uu, bi, :],
                                in_=t_sh[(b0 + bi) * P : (b0 + bi) * P + uu, :fagg],
                            )
                    if li < 3:
                        fnext = fh if li == 1 else fout
                        ev_dt = F8 if li == 1 else BF16
                        tnext = t2_sh if li == 1 else t3_sh
                        ev_g = pool.tile([P, G, fnext], ev_dt, tag="ev")

                    for bi in range(gb):
                        b = b0 + bi
                        u = used_rows(b)
                        kbb = int(kb[b])
                        soff = int(toff[b])

                        st_t = stpool.tile([P, kbb, P], g_dt, tag="st")
                        in0 = iota_t[:, :].unsqueeze(1).broadcast_to([P, kbb, P])
                        in1 = (
                            tn_t[:, soff : soff + kbb]
                            .unsqueeze(2)
                            .broadcast_to([P, kbb, P])
                        )
                        nc.vector.tensor_tensor(
                            out=st_t[:, :, :], in0=in0, in1=in1, op=AL.is_equal
                        )

                        # bias + self-loop: ident @ (b/deg^.5 + t_local_block)
                        bdt = pool.tile([P, fagg], BF16, tag="bdt")
                        nc.scalar.activation(
                            bdt[:, :], bb_t[:, :], AF.Copy,
                            scale=dinv_t[:, b : b + 1],
                        )
                        tlp = pool.tile([P, fagg], BF16, tag="tlp")
                        nc.vector.tensor_tensor(
                            out=tlp[:u, :], in0=tl_g[:u, bi, :], in1=bdt[:u, :],
                            op=AL.add,
                        )

                        psa = psp.tile([P, fagg], F32, tag="psa")
                        s = 0
                        for qq in range(NQ):
                            for sl in range(int(s_lo[b, qq]), int(s_hi[b, qq]) + 1):
                                nc.tensor.matmul(
                                    psa[:],
                                    lhsT=st_t[:, s, :],
                                    rhs=dsts[qq][:, sl, :fagg],
                                    start=(s == 0),
                                    stop=False,
                                )
                                s += 1
                        nc.tensor.matmul(
                            psa[:], lhsT=ident_t[:], rhs=tlp[:, :],
                            start=False, stop=True,
                        )
                        h_sb = pool.tile([P, fagg], BF16, tag="h_sb")
                        nc.scalar.activation(
                            h_sb[:u, :], psa[:u, :], AF.Relu, scale=dis_t[:u, b : b + 1]
                        )

                        if li < 3:
                            wx = w2x_t if li == 1 else w3x_t
                            fxw = fx1 if li == 1 else fx2
                            ps2 = psp.tile([P, fxw], F32, tag="ps2")
                            hT2 = pool.tile([P, 2, P], BF16, tag="hT2")
                            for cc in range(2):
                                pst = psp.tile([P, P], BF16, tag=f"pst{cc}")
                                nc.tensor.transpose(
                                    pst[:], h_sb[:, cc * P : (cc + 1) * P], ident_t[:]
                                )
                                nc.vector.tensor_copy(hT2[:, cc, :], pst[:])
                                nc.tensor.matmul(
                                    ps2[:u, :],
                                    lhsT=hT2[:, cc, :u],
                                    rhs=wx[:, cc, :fxw],
                                    start=(cc == 0),
                                    stop=(cc == 1),
                                )
                            nc.scalar.activation(
                                ev_g[:u, bi, :], ps2[:u, :fnext], AF.Copy,
                                scale=dis_t[:u, b : b + 1],
                            )
                            # fused projection partial: h{li} @ LW{li}
                            if li == 1:
                                nc.vector.tensor_tensor(
                                    out=acc_t[:u, b, :], in0=ps2[:u, fh:fx1],
                                    in1=lbbc[:u, :], op=AL.add,
                                )
                            else:
                                nc.vector.tensor_tensor(
                                    out=acc_t[:u, b, :], in0=ps2[:u, fout:fx2],
                                    in1=acc_t[:u, b, :], op=AL.add,
                                )
                        else:
                            ps3t = psp.tile([P, P], BF16, tag="pst0")
                            nc.tensor.transpose(ps3t[:fout, :], h_sb[:, :fout], ident_t[:])
                            h3T = pool.tile([fout, P], BF16, tag="hT0")
                            nc.vector.tensor_copy(h3T[:], ps3t[:fout, :])
                            pso = psp.tile([P, fout], F32, tag="ps2")
                            nc.tensor.matmul(
                                pso[:u, :], lhsT=h3T[:, :u], rhs=lw3_t[:, :],
                                start=True, stop=True,
                            )
                            # z = pso + acc (concat proj complete), kept in acc
                            nc.vector.tensor_tensor(
                                out=acc_t[:u, b, :], in0=pso[:u, :],
                                in1=acc_t[:u, b, :], op=AL.add,
                            )
                            nc.vector.tensor_reduce(
                                mbuf[:u, b : b + 1], acc_t[:u, b, :],
                                mybir.AxisListType.X, AL.max,
                            )
                            nm = pool.tile([P, 1], F32, tag="nm")
                            nc.vector.tensor_scalar(
                                out=nm[:u, :], in0=mbuf[:u, b : b + 1],
                                scalar1=-1.0, scalar2=None, op0=AL.mult,
                            )
                            e_t = pool.tile([P, fout], F32, tag="e_t")
                            nc.scalar.activation(
                                e_t[:u, :], acc_t[:u, b, :], AF.Exp,
                                bias=nm[:u, :1],
                                accum_out=ssbuf[:u, b : b + 1],
                            )

                    if li < 3:
                        if gw == gb * P:
                            nc.sync.dma_start(
                                out=tnext[b0 * P : b0 * P + gw, :fnext].rearrange(
                                    "(c k) f -> k c f", k=P
                                ),
                                in_=ev_g[:, :gb, :],
                            )
                        else:
                            for bi in range(gb):
                                uu = used_rows(b0 + bi)
                                nc.sync.dma_start(
                                    out=tnext[
                                        (b0 + bi) * P : (b0 + bi) * P + uu, :fnext
                                    ],
                                    in_=ev_g[:uu, bi, :],
                                )
                        for s4 in range(NQ):
                            if b0 <= int(s_end_blocks[s4]) < b0 + gb:
                                fire_ag(li + 1, s4)

                if li == 3:
                    ls_t = pool.tile([P, nb], F32, tag="ls_t")
                    nc.scalar.activation(ls_t[:, :], ssbuf[:, :], AF.Ln)
                    mls = pool.tile([P, nb], F32, tag="mls")
                    nc.vector.tensor_tensor(
                        out=mls[:, :], in0=mbuf[:, :], in1=ls_t[:, :], op=AL.add
                    )
                    nc.vector.tensor_tensor(
                        out=zf[:, :, :], in0=acc_t[:, :, :],
                        in1=mls[:, :].unsqueeze(2).broadcast_to([P, nb, fout]),
                        op=AL.subtract,
                    )
                    nc.sync.dma_start(
                        out=out_sh[0 : (nb - 1) * P, :].rearrange(
                            "(c k) f -> k c f", k=P
                        ),
                        in_=zf[:, : nb - 1, :],
                    )
                    lastu = S - (nb - 1) * P
                    nc.sync.dma_start(
                        out=out_sh[(nb - 1) * P :, :], in_=zf[:lastu, nb - 1, :]
                    )

            sc = nc.enter_named_scope("L1", False)
            layer(1, fh)
            nc.leave_named_scope("L1", sc[0], False)
            sc = nc.enter_named_scope("L2", False)
            layer(2, fh)
            nc.leave_named_scope("L2", sc[0], False)
            sc = nc.enter_named_scope("L3", False)
            layer(3, fo_pad)
            nc.leave_named_scope("L3", sc[0], False)

    nc.finalize()
    return nc


def kernel(x, edge_index, W1, b1, W2, b2, W3, b3, lin_w, lin_b):
    global LAST_EXEC_NS, LAST_SCOPES
    x = np.asarray(x)
    N = x.shape[0]
    S = N // NC
    fin, fh, fout = W1.shape[0], W2.shape[0], W3.shape[1]

    meta = _preprocess(np.asarray(edge_index, dtype=np.int64), N)
    nc = _build_program(meta, N, fin, fh, fout)

    dis = meta["dis"]
    nb = meta["n_blocks"]

    iota = np.tile(np.arange(P, dtype=np.float32), (P, 1)).astype(ml_dtypes.bfloat16)
    ident = np.eye(P, dtype=np.float32).astype(ml_dtypes.bfloat16)
    lbbc = np.tile(np.asarray(lin_b, np.float32), (P, 1)).astype(ml_dtypes.bfloat16)
    bb1 = np.tile(np.asarray(b1, np.float32), (P, 1)).astype(ml_dtypes.bfloat16)
    bb2 = np.tile(np.asarray(b2, np.float32), (P, 1)).astype(ml_dtypes.bfloat16)
    bb3 = np.tile(np.asarray(b3, np.float32), (P, 1)).astype(ml_dtypes.bfloat16)
    lw = np.asarray(lin_w, np.float32)
    w2x = np.concatenate([np.asarray(W2, np.float32), lw[:fh]], axis=1)
    w3x = np.concatenate([np.asarray(W3, np.float32), lw[fh : 2 * fh]], axis=1)
    lw3 = lw[2 * fh :]

    in_maps = []
    for c in range(NC):
        xs = np.asarray(x[c * S : (c + 1) * S], np.float32)
        dc = dis[c * S : (c + 1) * S]
        dis_blk = np.ones((P, nb), dtype=np.float32)
        for b in range(nb):
            u = min(P, S - b * P)
            dis_blk[:u, b] = dc[b * P : b * P + u]
        in_maps.append(
            {
                "xT": np.ascontiguousarray(xs.T).astype(ml_dtypes.bfloat16),
                "W1": np.asarray(W1, np.float32).astype(ml_dtypes.bfloat16),
                "W2X": w2x.astype(ml_dtypes.bfloat16),
                "W3X": w3x.astype(ml_dtypes.bfloat16),
                "LW3": lw3.astype(ml_dtypes.bfloat16),
                "idx16": meta["per_core"][c]["idx16"],
                "tn": meta["per_core"][c]["tn"],
                "iota": iota,
                "ident": ident,
                "disblk": dis_blk,
                "dinvblk": 1.0 / dis_blk,
                "bb1": bb1,
                "bb2": bb2,
                "bb3": bb3,
                "lbbc": lbbc,
            }
        )
    trace = bool(os.environ.get("GCN_TRACE"))
    res = run_bass_kernel_spmd(nc, in_maps, list(range(NC)), trace=trace)
    LAST_EXEC_NS = res.exec_time_ns
    LAST_SCOPES = res.per_core_scope_times
    out = np.concatenate([res.results[c]["out_sh"] for c in range(NC)], axis=0)
    return out.astype(np.float32)


# revision 57
# speedup vs baseline: 1.0202x; 1.0020x over previous
"""GCN 3-layer message-passing kernel for TRN2 (8 NeuronCores, SPMD) — v18.

v5 baseline (2.82 ms) was SWDGE-bound: 1176 dma_gathers + 1176 reg_loads
saturated the Pool engine; the hard wall is SWDGE descriptor generation
(~2.5 ns/descriptor aggregate, 256 B/edge minimum). v18 (1.92 ms):
  - Gathers batched G=4 blocks per call per queue (300 calls), with
    EXACT-call idx packing: per-core edge streams concatenated with no
    per-block slot alignment (only the call tail is 0-padded), cutting
    descriptors 238k -> 211k per layer. A block's matmul schedule covers
    the union (over cores) of slots its edges touch at core-dependent
    offsets; foreign edges in shared slots hit all-zero one-hot columns.
    Compile-time-constant counts (no reg_loads); ~2-3k descriptors/call
    pipelines through the SWDGE rings without head-of-line blocking;
    gather pool bufs=5, st-build pool bufs=4 (pre-builds one-hot tiles
    through AllGather-gated stalls).
  - Self-loops removed from the gather/scatter path; their contribution
    (the local t-block) rides the existing per-block bias matmul:
    ident @ (b/sqrt(deg) + t_local), bias scaled on the scalar engine.
  - Final projection fused into the L1/L2 transforms (rhs widened to
    [W2 | LW1] / [W3 | LW2]) with an SBUF accumulator; h1T/h2T DRAM
    roundtrip and L3's extra matmuls/loads deleted.
  - Slices [26,26,26,20]: near-equal sizes balance the per-queue gather
    load under the (race-free) 1:1 slice->queue mapping, while the
    smaller last slice still shrinks the inter-layer AllGather bubble.
  - Per-group (512-row) batched DMAs for xT/t-table/t-local/output
    (HWDGE dispatch costs ~0.9 us of issuing-engine time each).
  - L3 log-softmax: Exp in-loop (accumulated row sums), single batched
    Ln + subtract + 2 output DMAs at the end (no act-table thrash).
"""

import os
import sys

sys.path.insert(0, "/opt/trn_rl_repo")

import numpy as np
import ml_dtypes

import concourse.bass as bass  # noqa: F401
import concourse.mybir as mybir
import concourse.tile as tile
from concourse import bacc
from concourse._compat import cdiv
from concourse.bass_utils import run_bass_kernel_spmd

F32 = mybir.dt.float32
F8 = mybir.dt.float8e4
BF16 = mybir.dt.bfloat16
I16 = mybir.dt.int16
I32 = mybir.dt.int32
AL = mybir.AluOpType
AF = mybir.ActivationFunctionType

NC = 8
P = 128
G = 5  # blocks per gather call
SLICE_BLOCKS = [26, 26, 26, 20]  # 98 blocks; balanced queues, smallish final AG
NQ = 4

LAST_EXEC_NS = None
LAST_SCOPES = None


def _cdiv_np(a, b):
    return (a + b - 1) // b


def _group_cumcount(grp: np.ndarray) -> np.ndarray:
    n = len(grp)
    if n == 0:
        return np.zeros(0, dtype=np.int64)
    is_new = np.ones(n, dtype=bool)
    is_new[1:] = grp[1:] != grp[:-1]
    idx = np.arange(n)
    start = np.maximum.accumulate(np.where(is_new, idx, 0))
    return idx - start


def _slice_geometry(S):
    ends = np.cumsum(SLICE_BLOCKS) * P
    starts = np.concatenate([[0], ends[:-1]])
    ends = np.minimum(ends, S)
    rows = ends - starts
    return starts, ends, rows


def _preprocess(edge_index: np.ndarray, n_nodes: int):
    N = n_nodes
    S = N // NC
    n_blocks = cdiv(S, P)
    assert sum(SLICE_BLOCKS) == n_blocks
    n_groups = cdiv(n_blocks, G)
    sstart, send, srows = _slice_geometry(S)
    assert all(r * NC <= 32767 for r in srows)

    src = edge_index[0]
    dst = edge_index[1]
    # deg includes self-loops (reference adds them); edges here exclude them.
    deg = (np.bincount(dst, minlength=N) + 1).astype(np.float64)
    dis = (1.0 / np.sqrt(deg)).astype(np.float32)

    core = dst // S
    block = (dst % S) // P
    t_local = (dst % S) % P

    sc = src // S
    sloc = src % S
    q = np.searchsorted(send, sloc, side="right")
    src_local = sc * srows[q] + (sloc - sstart[q])

    counts = np.zeros((NC, n_blocks, NQ), dtype=np.int64)
    np.add.at(counts, (core, block, q), 1)

    # Exact-call packing: each (group, q) call's idx stream is the per-core
    # concatenation of its blocks' edges with NO per-block slot alignment;
    # only the call tail is 0-padded to a slot boundary. A block's edges can
    # straddle slots at core-dependent offsets, so its matmul schedule covers
    # the UNION (over cores) of touched slots [s_lo, s_hi]; cores without
    # edges in a given slot leave all-zero one-hot columns there.
    ofs = np.zeros((NC, n_blocks, NQ), dtype=np.int64)
    call_slots = np.zeros((n_groups, NQ), dtype=np.int64)
    call_base = np.zeros((n_groups, NQ), dtype=np.int64)  # slot units
    s_lo = np.zeros((n_blocks, NQ), dtype=np.int64)
    s_hi = np.full((n_blocks, NQ), -1, dtype=np.int64)
    acc = 0
    for g in range(n_groups):
        bs = list(range(g * G, min((g + 1) * G, n_blocks)))
        for qq in range(NQ):
            c = counts[:, bs, qq]  # [NC, gb]
            o = np.cumsum(c, axis=1) - c
            ofs[:, bs, qq] = o
            cs = max(1, _cdiv_np(int(c.sum(axis=1).max()), P))
            call_base[g, qq] = acc
            call_slots[g, qq] = cs
            acc += cs
            for i, b in enumerate(bs):
                has = c[:, i] > 0
                if has.any():
                    s_lo[b, qq] = int((o[has, i] // P).min())
                    s_hi[b, qq] = int(((o[has, i] + c[has, i] - 1) // P).max())
    K_total = acc  # gather slots across all calls
    IW = K_total * 8

    ucols = s_hi - s_lo + 1  # [n_blocks, NQ], >= 0
    kb = ucols.sum(axis=1)
    assert (kb >= 1).all()
    toff = np.concatenate([[0], np.cumsum(kb)[:-1]])
    qoff = toff[:, None] + np.concatenate(
        [np.zeros((n_blocks, 1), np.int64), np.cumsum(ucols, axis=1)[:, :-1]], axis=1
    )  # [n_blocks, NQ]
    K_st = int(kb.sum())

    order = np.lexsort((src, q, block, core))
    so_sl = src_local[order]
    so_tl = t_local[order]
    so_core = core[order]
    so_block = block[order]
    so_q = q[order]

    per_core = []
    for c in range(NC):
        m = so_core == c
        cb, cq = so_block[m], so_q[m]
        csl, ctl = so_sl[m], so_tl[m]
        grp = cb * NQ + cq
        pos = _group_cumcount(grp)

        idx16 = np.zeros((16, IW), dtype=np.int16)  # pad = row 0
        tn = np.full((P, K_st), -1.0, dtype=np.float32)  # pad -> zero col

        cpos = ofs[c, cb, cq] + pos  # position within the (g,q) call
        epos = call_base[cb // G, cq] * P + cpos  # global idx entry
        idx16[epos % 16, epos // 16] = csl.astype(np.int16)
        tn[cpos % P, qoff[cb, cq] + cpos // P - s_lo[cb, cq]] = ctl.astype(
            np.float32
        )

        per_core.append(
            {
                "idx16": np.tile(idx16, (8, 1)),
                "tn": tn.astype(ml_dtypes.bfloat16),
            }
        )

    return {
        "s_lo": s_lo,
        "s_hi": s_hi,
        "kb": kb,
        "toff": toff,
        "K_total": K_total,
        "K_st": K_st,
        "IW": IW,
        "call_base": call_base,
        "call_slots": call_slots,
        "n_blocks": n_blocks,
        "n_groups": n_groups,
        "srows": srows,
        "per_core": per_core,
        "dis": dis,
    }


def _build_program(meta, n_nodes, fin, fh, fout):
    N = n_nodes
    S = N // NC
    nb = meta["n_blocks"]
    ng = meta["n_groups"]
    srows = meta["srows"]
    s_lo = meta["s_lo"]
    s_hi = meta["s_hi"]
    kb = meta["kb"]
    toff = meta["toff"]
    K_total = meta["K_total"]
    K_st = meta["K_st"]
    IW = meta["IW"]
    call_base = meta["call_base"]
    call_slots = meta["call_slots"]
    fo_pad = 128
    fx1 = fh + fout  # [W2 | LW1] width
    fx2 = fout + fout  # [W3 | LW2] width
    sstart = np.concatenate([[0], np.cumsum(SLICE_BLOCKS)[:-1]]) * P
    s_end_blocks = np.cumsum(SLICE_BLOCKS) - 1
    max_cs_q = call_slots.max(axis=0)  # per-q dst tile sizes
    # Static greedy queue balancing: slices are uneven (tiny last slice),
    # so assign each call to the least-loaded SWDGE queue instead of 1:1.
    qload = [0] * NQ
    queue_of = {}
    for g in range(ng):
        for qq in range(NQ):
            k = min(range(NQ), key=lambda i: qload[i])
            queue_of[(g, qq)] = k
            qload[k] += int(call_slots[g, qq])

    nc = bacc.Bacc(num_swdge_queues=NQ)

    xT = nc.dram_tensor("xT", [fin, S], BF16, kind="ExternalInput")
    W1 = nc.dram_tensor("W1", [fin, fh], BF16, kind="ExternalInput")
    W2X = nc.dram_tensor("W2X", [fh, fx1], BF16, kind="ExternalInput")
    W3X = nc.dram_tensor("W3X", [fh, fx2], BF16, kind="ExternalInput")
    LW3 = nc.dram_tensor("LW3", [fout, fout], BF16, kind="ExternalInput")
    idx16 = nc.dram_tensor("idx16", [P, IW], I16, kind="ExternalInput")
    tn_in = nc.dram_tensor("tn", [P, K_st], BF16, kind="ExternalInput")
    iota_in = nc.dram_tensor("iota", [P, P], BF16, kind="ExternalInput")
    ident_in = nc.dram_tensor("ident", [P, P], BF16, kind="ExternalInput")
    dis_in = nc.dram_tensor("disblk", [P, nb], F32, kind="ExternalInput")
    dinv_in = nc.dram_tensor("dinvblk", [P, nb], F32, kind="ExternalInput")
    bb1_in = nc.dram_tensor("bb1", [P, fh], BF16, kind="ExternalInput")
    bb2_in = nc.dram_tensor("bb2", [P, fh], BF16, kind="ExternalInput")
    bb3_in = nc.dram_tensor("bb3", [P, fout], BF16, kind="ExternalInput")
    lbbc_in = nc.dram_tensor("lbbc", [P, fout], BF16, kind="ExternalInput")
    out_sh = nc.dram_tensor("out_sh", [S, fout], F32, kind="ExternalOutput")

    t1_sh = nc.dram_tensor("t1_sh", [S, fh], F8)
    t2_sh = nc.dram_tensor("t2_sh", [S, fh], F8)
    t3_sh = nc.dram_tensor("t3_sh", [S, fo_pad], BF16)

    tabs = {}
    tab_dt = {1: F8, 2: F8, 3: BF16}
    for li, fel in ((1, fh), (2, fh), (3, fo_pad)):
        tabs[li] = tuple(
            nc.dram_tensor(f"t{li}S{s}", [int(srows[s]) * NC, fel], tab_dt[li],
                           addr_space="Shared")
            for s in range(NQ)
        )
    t_shs = {1: t1_sh, 2: t2_sh, 3: t3_sh}

    rg = [list(range(NC))]

    def used_rows(b):
        return min(P, S - b * P)

    def fire_ag(li, s):
        t_sh = t_shs[li]
        r0, r1 = int(sstart[s]), int(sstart[s]) + int(srows[s])
        sc = nc.enter_named_scope(f"ag{li}S{s}", False)
        nc.gpsimd.collective_compute(
            "AllGather", AL.bypass, ins=[t_sh[r0:r1, :]], outs=[tabs[li][s][:, :]],
            replica_groups=rg,
        )
        nc.leave_named_scope(f"ag{li}S{s}", sc[0], False)

    with tile.TileContext(nc) as tc:
        with (
            tc.tile_pool(name="const", bufs=1) as cpool,
            tc.tile_pool(name="sb", bufs=3) as pool,
            tc.tile_pool(name="gath", bufs=4) as gpool,
            tc.tile_pool(name="st", bufs=4) as stpool,
            tc.tile_pool(name="xg", bufs=2) as xgpool,
            tc.tile_pool(name="ps", bufs=2, space="PSUM") as psp,
        ):
            iota_t = cpool.tile([P, P], BF16)
            nc.sync.dma_start(out=iota_t[:], in_=iota_in[:, :])
            ident_t = cpool.tile([P, P], BF16)
            nc.sync.dma_start(out=ident_t[:], in_=ident_in[:, :])
            w1_t = cpool.tile([P, 2, fh], BF16)
            nc.sync.dma_start(out=w1_t[:], in_=W1[:, :].rearrange("(c k) f -> k c f", k=P))
            w2x_t = cpool.tile([P, 2, fx1], BF16)
            nc.sync.dma_start(out=w2x_t[:], in_=W2X[:, :].rearrange("(c k) f -> k c f", k=P))
            w3x_t = cpool.tile([P, 2, fx2], BF16)
            nc.sync.dma_start(out=w3x_t[:], in_=W3X[:, :].rearrange("(c k) f -> k c f", k=P))
            lw3_t = cpool.tile([fout, fout], BF16)
            nc.sync.dma_start(out=lw3_t[:], in_=LW3[:, :])
            lbbc = cpool.tile([P, fout], BF16)
            nc.sync.dma_start(out=lbbc[:], in_=lbbc_in[:, :])
            dis_t = cpool.tile([P, nb], F32)
            nc.sync.dma_start(out=dis_t[:], in_=dis_in[:, :])
            dinv_t = cpool.tile([P, nb], F32)
            nc.sync.dma_start(out=dinv_t[:], in_=dinv_in[:, :])
            bb_ts = {}
            for li, (bb_in, fel) in enumerate(
                ((bb1_in, fh), (bb2_in, fh), (bb3_in, fout)), start=1
            ):
                bb_t = cpool.tile([P, fel], BF16)
                nc.sync.dma_start(out=bb_t[:], in_=bb_in[:, :])
                bb_ts[li] = bb_t
            idx_t = cpool.tile([P, IW], I16)
            nc.sync.dma_start(out=idx_t[:], in_=idx16[:, :])
            tn_t = cpool.tile([P, K_st], BF16)
            nc.sync.dma_start(out=tn_t[:], in_=tn_in[:, :])
            acc_t = cpool.tile([P, nb, fout], F32)  # fused projection accum
            zf = cpool.tile([P, nb, fout], F32)  # final log-probs
            mbuf = cpool.tile([P, nb], F32)
            ssbuf = cpool.tile([P, nb], F32)
            nc.vector.memset(mbuf[:], 0.0)
            nc.vector.memset(ssbuf[:], 1.0)

            sc_T = nc.enter_named_scope("phaseT", False)
            for g in range(ng):
                b0 = g * G
                gb = min(G, nb - b0)
                gw = min(G * P, S - b0 * P)
                xg = xgpool.tile([P, 2, G * P], BF16, tag="xg")
                for cc in range(2):
                    nc.sync.dma_start(
                        out=xg[:, cc, :gw],
                        in_=xT[cc * P : (cc + 1) * P, b0 * P : b0 * P + gw],
                    )
                ev_g = pool.tile([P, G, fh], F8, tag="evq")
                for bi in range(gb):
                    b = b0 + bi
                    u = used_rows(b)
                    ps1 = psp.tile([P, fh], F32, tag="ps2")
                    for cc in range(2):
                        nc.tensor.matmul(
                            ps1[:u, :],
                            lhsT=xg[:, cc, bi * P : bi * P + u],
                            rhs=w1_t[:, cc, :],
                            start=(cc == 0),
                            stop=(cc == 1),
                        )
                    nc.scalar.activation(
                        ev_g[:u, bi, :], ps1[:u, :], AF.Copy,
                        scale=dis_t[:u, b : b + 1],
                    )
                if gw == gb * P:
                    nc.sync.dma_start(
                        out=t1_sh[b0 * P : b0 * P + gw, :].rearrange(
                            "(c k) f -> k c f", k=P
                        ),
                        in_=ev_g[:, :gb, :],
                    )
                else:
                    for bi in range(gb):
                        uu = used_rows(b0 + bi)
                        nc.sync.dma_start(
                            out=t1_sh[(b0 + bi) * P : (b0 + bi) * P + uu, :],
                            in_=ev_g[:uu, bi, :],
                        )
                for s in range(NQ):
                    if b0 <= int(s_end_blocks[s]) < b0 + gb:
                        fire_ag(1, s)
            nc.leave_named_scope("phaseT", sc_T[0], False)

            def layer(li, felem):
                fagg = fh if li < 3 else fout
                g_dt = F8 if li < 3 else BF16
                bb_t = bb_ts[li]
                t_sh = t_shs[li]
                for g in range(ng):
                    b0 = g * G
                    gb = min(G, nb - b0)
                    dsts = []
                    for qq in range(NQ):
                        cs = int(call_slots[g, qq])
                        cbase = int(call_base[g, qq])
                        tab = tabs[li][qq]
                        rows = int(srows[qq]) * NC
                        dst = gpool.tile(
                            [P, int(max_cs_q[qq]), felem], g_dt, tag=f"dst{qq}"
                        )
                        nc.gpsimd.dma_gather(
                            dst[:, :cs, :],
                            tab[0:rows, :],
                            idx_t[:, cbase * 8 : (cbase + cs) * 8],
                            cs * P,
                            cs * P,
                            felem,
                            single_packet=False,
                            queue_num=queue_of[(g, qq)],
                        )
                        dsts.append(dst)

                    gw = min(G * P, S - b0 * P)
                    tl_g = pool.tile([P, G, fagg], g_dt, tag="tl")
                    if gw == gb * P:
                        nc.scalar.dma_start(
                            out=tl_g[:, :gb, :],
                            in_=t_sh[b0 * P : b0 * P + gw, :fagg].rearrange(
                                "(c k) f -> k c f", k=P
                            ),
                        )
                    else:
                        for bi in range(gb):
                            uu = used_rows(b0 + bi)
                            nc.scalar.dma_start(
                                out=tl_g[:uu, bi, :],
                                in_=t_sh[(b0 + bi) * P : (b0 + bi) * P + uu, :fagg],
                            )
                    if li < 3:
                        fnext = fh if li == 1 else fout
                        ev_dt = F8 if li == 1 else BF16
                        tnext = t2_sh if li == 1 else t3_sh
                        ev_g = pool.tile([P, G, fnext], ev_dt, tag="ev")

                    for bi in range(gb):
                        b = b0 + bi
                        u = used_rows(b)
                        kbb = int(kb[b])
                        soff = int(toff[b])

                        st_t = stpool.tile([P, kbb, P], g_dt, tag="st")
                        in0 = iota_t[:, :].unsqueeze(1).broadcast_to([P, kbb, P])
                        in1 = (
                            tn_t[:, soff : soff + kbb]
                            .unsqueeze(2)
                            .broadcast_to([P, kbb, P])
                        )
                        nc.vector.tensor_tensor(
                            out=st_t[:, :, :], in0=in0, in1=in1, op=AL.is_equal
                        )

                        # bias + self-loop: ident @ (b/deg^.5 + t_local_block)
                        bdt = pool.tile([P, fagg], BF16, tag="bdt")
                        nc.scalar.activation(
                            bdt[:, :], bb_t[:, :], AF.Copy,
                            scale=dinv_t[:, b : b + 1],
                        )
                        tlp = pool.tile([P, fagg], BF16, tag="tlp")
                        nc.vector.tensor_tensor(
                            out=tlp[:u, :], in0=tl_g[:u, bi, :], in1=bdt[:u, :],
                            op=AL.add,
                        )

                        psa = psp.tile([P, fagg], F32, tag="psa")
                        s = 0
                        for qq in range(NQ):
                            for sl in range(int(s_lo[b, qq]), int(s_hi[b, qq]) + 1):
                                nc.tensor.matmul(
                                    psa[:],
                                    lhsT=st_t[:, s, :],
                                    rhs=dsts[qq][:, sl, :fagg],
                                    start=(s == 0),
                                    stop=False,
                                )
                                s += 1
                        nc.tensor.matmul(
                            psa[:], lhsT=ident_t[:], rhs=tlp[:, :],
                            start=False, stop=True,
                        )
                        h_sb = pool.tile([P, fagg], BF16, tag="h_sb")
                        nc.scalar.activation(
                            h_sb[:u, :], psa[:u, :], AF.Relu, scale=dis_t[:u, b : b + 1]
                        )

                        if li < 3:
                            wx = w2x_t if li == 1 else w3x_t
                            fxw = fx1 if li == 1 else fx2
                            ps2 = psp.tile([P, fxw], F32, tag="ps2")
                            hT2 = pool.tile([P, 2, P], BF16, tag="hT2")
                            for cc in range(2):
                                pst = psp.tile([P, P], BF16, tag=f"pst{cc}")
                                nc.tensor.transpose(
                                    pst[:], h_sb[:, cc * P : (cc + 1) * P], ident_t[:]
                                )
                                nc.vector.tensor_copy(hT2[:, cc, :], pst[:])
                                nc.tensor.matmul(
                                    ps2[:u, :],
                                    lhsT=hT2[:, cc, :u],
                                    rhs=wx[:, cc, :fxw],
                                    start=(cc == 0),
                                    stop=(cc == 1),
                                )
                            nc.scalar.activation(
                                ev_g[:u, bi, :], ps2[:u, :fnext], AF.Copy,
                                scale=dis_t[:u, b : b + 1],
                            )
                            # fused projection partial: h{li} @ LW{li}
                            if li == 1:
                                nc.vector.tensor_tensor(
                                    out=acc_t[:u, b, :], in0=ps2[:u, fh:fx1],
                                    in1=lbbc[:u, :], op=AL.add,
                                )
                            else:
                                nc.vector.tensor_tensor(
                                    out=acc_t[:u, b, :], in0=ps2[:u, fout:fx2],
                                    in1=acc_t[:u, b, :], op=AL.add,
                                )
                        else:
                            ps3t = psp.tile([P, P], BF16, tag="pst0")
                            nc.tensor.transpose(ps3t[:fout, :], h_sb[:, :fout], ident_t[:])
                            h3T = pool.tile([fout, P], BF16, tag="hT0")
                            nc.vector.tensor_copy(h3T[:], ps3t[:fout, :])
                            pso = psp.tile([P, fout], F32, tag="ps2")
                            nc.tensor.matmul(
                                pso[:u, :], lhsT=h3T[:, :u], rhs=lw3_t[:, :],
                                start=True, stop=True,
                            )
                            # z = pso + acc (concat proj complete), kept in acc
                            nc.vector.tensor_tensor(
                                out=acc_t[:u, b, :], in0=pso[:u, :],
                                in1=acc_t[:u, b, :], op=AL.add,
                            )
                            nc.vector.tensor_reduce(
                                mbuf[:u, b : b + 1], acc_t[:u, b, :],
                                mybir.AxisListType.X, AL.max,
                            )
                            nm = pool.tile([P, 1], F32, tag="nm")
                            nc.vector.tensor_scalar(
                                out=nm[:u, :], in0=mbuf[:u, b : b + 1],
                                scalar1=-1.0, scalar2=None, op0=AL.mult,
                            )
                            e_t = pool.tile([P, fout], F32, tag="e_t")
                            nc.scalar.activation(
                                e_t[:u, :], acc_t[:u, b, :], AF.Exp,
                                bias=nm[:u, :1],
                                accum_out=ssbuf[:u, b : b + 1],
                            )

                    if li < 3:
                        if gw == gb * P:
                            nc.sync.dma_start(
                                out=tnext[b0 * P : b0 * P + gw, :fnext].rearrange(
                                    "(c k) f -> k c f", k=P
                                ),
                                in_=ev_g[:, :gb, :],
                            )
                        else:
                            for bi in range(gb):
                                uu = used_rows(b0 + bi)
                                nc.sync.dma_start(
                                    out=tnext[
                                        (b0 + bi) * P : (b0 + bi) * P + uu, :fnext
                                    ],
                                    in_=ev_g[:uu, bi, :],
                                )
                        for s4 in range(NQ):
                            if b0 <= int(s_end_blocks[s4]) < b0 + gb:
                                fire_ag(li + 1, s4)

                if li == 3:
                    ls_t = pool.tile([P, nb], F32, tag="ls_t")
                    nc.scalar.activation(ls_t[:, :], ssbuf[:, :], AF.Ln)
                    mls = pool.tile([P, nb], F32, tag="mls")
                    nc.vector.tensor_tensor(
                        out=mls[:, :], in0=mbuf[:, :], in1=ls_t[:, :], op=AL.add
                    )
                    nc.vector.tensor_tensor(
                        out=zf[:, :, :], in0=acc_t[:, :, :],
                        in1=mls[:, :].unsqueeze(2).broadcast_to([P, nb, fout]),
                        op=AL.subtract,
                    )
                    nc.sync.dma_start(
                        out=out_sh[0 : (nb - 1) * P, :].rearrange(
                            "(c k) f -> k c f", k=P
                        ),
                        in_=zf[:, : nb - 1, :],
                    )
                    lastu = S - (nb - 1) * P
                    nc.sync.dma_start(
                        out=out_sh[(nb - 1) * P :, :], in_=zf[:lastu, nb - 1, :]
                    )

            sc = nc.enter_named_scope("L1", False)
            layer(1, fh)
            nc.leave_named_scope("L1", sc[0], False)
            sc = nc.enter_named_scope("L2", False)
            layer(2, fh)
            nc.leave_named_scope("L2", sc[0], False)
            sc = nc.enter_named_scope("L3", False)
            layer(3, fo_pad)
            nc.leave_named_scope("L3", sc[0], False)

    nc.finalize()
    return nc


def kernel(x, edge_index, W1, b1, W2, b2, W3, b3, lin_w, lin_b):
    global LAST_EXEC_NS, LAST_SCOPES
    x = np.asarray(x)
    N = x.shape[0]
    S = N // NC
    fin, fh, fout = W1.shape[0], W2.shape[0], W3.shape[1]

    meta = _preprocess(np.asarray(edge_index, dtype=np.int64), N)
    nc = _build_program(meta, N, fin, fh, fout)

    dis = meta["dis"]
    nb = meta["n_blocks"]

    iota = np.tile(np.arange(P, dtype=np.float32), (P, 1)).astype(ml_dtypes.bfloat16)
    ident = np.eye(P, dtype=np.float32).astype(ml_dtypes.bfloat16)
    lbbc = np.tile(np.asarray(lin_b, np.float32), (P, 1)).astype(ml_dtypes.bfloat16)
    bb1 = np.tile(np.asarray(b1, np.float32), (P, 1)).astype(ml_dtypes.bfloat16)
    bb2 = np.tile(np.asarray(b2, np.float32), (P, 1)).astype(ml_dtypes.bfloat16)
    bb3 = np.tile(np.asarray(b3, np.float32), (P, 1)).astype(ml_dtypes.bfloat16)
    lw = np.asarray(lin_w, np.float32)
    w2x = np.concatenate([np.asarray(W2, np.float32), lw[:fh]], axis=1)
    w3x = np.concatenate([np.asarray(W3, np.float32), lw[fh : 2 * fh]], axis=1)
    lw3 = lw[2 * fh :]

    in_maps = []
    for c in range(NC):
        xs = np.asarray(x[c * S : (c + 1) * S], np.float32)
        dc = dis[c * S : (c + 1) * S]
        dis_blk = np.ones((P, nb), dtype=np.float32)
        for b in range(nb):
            u = min(P, S - b * P)
            dis_blk[:u, b] = dc[b * P : b * P + u]
        in_maps.append(
            {
                "xT": np.ascontiguousarray(xs.T).astype(ml_dtypes.bfloat16),
                "W1": np.asarray(W1, np.float32).astype(ml_dtypes.bfloat16),
                "W2X": w2x.astype(ml_dtypes.bfloat16),
                "W3X": w3x.astype(ml_dtypes.bfloat16),
                "LW3": lw3.astype(ml_dtypes.bfloat16),
                "idx16": meta["per_core"][c]["idx16"],
                "tn": meta["per_core"][c]["tn"],
                "iota": iota,
                "ident": ident,
                "disblk": dis_blk,
                "dinvblk": 1.0 / dis_blk,
                "bb1": bb1,
                "bb2": bb2,
                "bb3": bb3,
                "lbbc": lbbc,
            }
        )
    trace = bool(os.environ.get("GCN_TRACE"))
    res = run_bass_kernel_spmd(nc, in_maps, list(range(NC)), trace=trace)
    LAST_EXEC_NS = res.exec_time_ns
    LAST_SCOPES = res.per_core_scope_times
    out = np.concatenate([res.results[c]["out_sh"] for c in range(NC)], axis=0)
    return out.astype(np.float32)
